# revision 1
# baseline (speedup 1.0000x reference)
"""GNN message-passing kernel for Trainium2 (8 NeuronCores, batch-parallel).

Computation (per reference):
    norm_adj = adjacency * dinv * dinv.T + I            [10,10]   (host, O(100) flops)
    support  = einsum('bcf,fo->bco', x, kernel)         [B,C,O]
    out      = elu(einsum('ij,bjo->bio', norm_adj, support) + bias)
    out      = (out - mean) * rsqrt(var+eps) * gamma + beta

Device strategy per core (512 batches = 5120 rows of [b,c] x f):
  1. "Transposing mix": PE matmul with x-chunks [crows<=120, 128f] as the
     stationary operand and a block-diagonal norm_adj matrix [crows, 256pad]
     as the moving operand. One op both applies the channel mix and lands
     the activations transposed ([f, rows]) as needed by the main matmul.
  2. Main matmul, fp32r full-rate: outT[o,rows] += K[f,o].T @ yT[f,rows],
     kernel matrix resident in SBUF.
  3. Epilogue on ACT/DVE with per-partition (o) params:
     elu(z) = min(exp(z), relu(z)+1) - 1 (exact), then folded BN affine.
     Output stored transposed [O, rows]; host transposes while unsharding.
"""

from contextlib import ExitStack

import numpy as np

import concourse.bass as bass
import concourse.bacc as bacc
import concourse.mybir as mybir
import concourse.tile as tile
from concourse.bass_utils import run_bass_kernel_spmd

F32 = mybir.dt.float32
F32R = mybir.dt.float32r
ALU = mybir.AluOpType
ACTF = mybir.ActivationFunctionType

P = 128
BN_EPS = 1e-3
N_CORES = 8
C = 10  # channels
BDW = 256  # mix moving-operand pad width


def build_nc(rows, F, O, chunk_batches=(12, 12, 8), n_cores=N_CORES, repeats=1):
    """Build the per-core Bass program. rows = local (b,c) rows, F/O = feat dims.

    repeats>1 replays the whole computation (for timing amplification)."""
    panel = sum(chunk_batches) * C  # rows per panel (multiple of 10, >=256)
    assert rows % panel == 0
    n_panels = rows // panel
    FT, OT = F // P, O // P
    bd_sizes = sorted({nb * C for nb in chunk_batches})

    nc = bacc.Bacc(
        "TRN2",
        target_bir_lowering=False,
        debug=False,
        enable_asserts=False,
        num_devices=n_cores,
    )
    x_d = nc.dram_tensor("x_local", [rows, F], F32, kind="ExternalInput").ap()
    k_d = nc.dram_tensor("kern", [F, O], F32, kind="ExternalInput").ap()
    # blob packs the small constants into one DMA: per bd size 256 cols, then
    # prm cols [0:OT]=bias_t, [OT:2OT]=scale_t, [2OT:3OT]=shift2_t (per-partition o)
    blob_cols = BDW * len(bd_sizes) + 3 * OT
    blob_d = nc.dram_tensor("blob", [P, blob_cols], F32, kind="ExternalInput").ap()
    outT_d = nc.dram_tensor("outT", [O, rows], F32, kind="ExternalOutput").ap()

    with tile.TileContext(nc) as tc, ExitStack() as ctx:
        const = ctx.enter_context(tc.tile_pool(name="const", bufs=1))
        blob = const.tile([P, blob_cols], F32R, name="blob")
        nc.sync.dma_start(blob, blob_d.bitcast(F32R))
        bd_t = {
            sz: blob[:sz, BDW * i : BDW * (i + 1)]
            for i, sz in enumerate(bd_sizes)
        }
        prm = blob[:, BDW * len(bd_sizes) :].bitcast(F32)
        kb = [const.tile([P, O], F32R, name=f"kb{fb}", tag=f"kb{fb}") for fb in range(FT)]
        for fb in range(FT):
            nc.scalar.dma_start(kb[fb], k_d[fb * P : (fb + 1) * P, :].bitcast(F32R))

        xpool = ctx.enter_context(tc.tile_pool(name="xpool", bufs=3))
        ypool = ctx.enter_context(tc.tile_pool(name="ypool", bufs=1))
        mixps = ctx.enter_context(tc.tile_pool(name="mixps", bufs=2, space="PSUM"))
        mainps = ctx.enter_context(tc.tile_pool(name="mainps", bufs=4, space="PSUM"))
        tmp = ctx.enter_context(tc.tile_pool(name="tmp", bufs=2))

        for rep in range(repeats):
          for pi in range(n_panels):
            row0 = pi * panel
            ytall = ypool.tile([P, FT, panel], F32R, name=f"r{rep}_yt_{pi}", tag="yt")
            # ---- mix phase: yT[f, rows_panel] = blockdiag(normadj) applied to x
            coff = 0
            for ci, nb in enumerate(chunk_batches):
                crows = nb * C
                xt = xpool.tile([120, F], F32R, name=f"r{rep}_x_{pi}_{ci}", tag="xc")[:crows]
                nc.sync.dma_start(xt, x_d[row0 + coff : row0 + coff + crows, :].bitcast(F32R))
                for fbp in range(FT // 4):
                    fb = 4 * fbp
                    ps = mixps.tile([P, 4, BDW], F32, name=f"r{rep}_mps_{pi}_{ci}_{fbp}", tag="mixps")
                    for q in range(4):
                        nc.tensor.matmul(
                            ps[:, q, :],
                            lhsT=xt[:, (fb + q) * P : (fb + q + 1) * P],
                            rhs=bd_t[crows],
                            start=True,
                            stop=True,
                        )
                    nc.vector.tensor_copy(
                        ytall[:, fb : fb + 4, coff : coff + crows], ps[:, :, :crows]
                    )
                coff += crows
            # ---- main matmul + epilogue per o-tile
            for ot in range(OT):
                ps = mainps.tile([P, panel], F32, name=f"r{rep}_ops_{pi}_{ot}", tag="mainps")
                for fb in range(FT):
                    nc.tensor.matmul(
                        ps,
                        lhsT=kb[fb][:, ot * P : (ot + 1) * P],
                        rhs=ytall[:, fb, :],
                        start=(fb == 0),
                        stop=(fb == FT - 1),
                    )
                bias_ap = prm[:, ot : ot + 1]
                scale_ap = prm[:, OT + ot : OT + ot + 1]
                shift_ap = prm[:, 2 * OT + ot : 2 * OT + ot + 1]
                e = tmp.tile([P, panel], F32, name=f"r{rep}_e_{pi}_{ot}", tag="e")
                t0 = tmp.tile([P, panel], F32, name=f"r{rep}_t0_{pi}_{ot}", tag="t0")
                s = tmp.tile([P, panel], F32, name=f"r{rep}_s_{pi}_{ot}", tag="s")
                fin = tmp.tile([P, panel], F32, name=f"r{rep}_fin_{pi}_{ot}", tag="fin")
                nc.scalar.activation(e, ps, ACTF.Exp, bias=bias_ap)
                nc.scalar.activation(t0, ps, ACTF.Relu, bias=bias_ap)
                # elu(zb) + 1 = min(exp(zb), relu(zb) + 1)   (exact identity)
                nc.vector.scalar_tensor_tensor(
                    s, in0=t0, scalar=1.0, in1=e, op0=ALU.add, op1=ALU.min
                )
                # fin = s*scale + (shift - scale) = elu*scale + shift
                nc.vector.tensor_scalar(
                    fin, s, scale_ap, shift_ap, op0=ALU.mult, op1=ALU.add
                )
                nc.scalar.dma_start(outT_d[ot * P : (ot + 1) * P, row0 : row0 + panel], fin)
    nc.compile()
    return nc


def _host_prep(adjacency, kern, bias, gamma, beta, moving_mean, moving_var,
               chunk_batches=(12, 12, 8), O=2048):
    """Build the tiny derived inputs on the host."""
    A = np.asarray(adjacency, np.float32)
    deg = np.maximum(np.abs(A).sum(axis=1, keepdims=True), 1e-8)
    dinv = deg ** -0.5
    na = A * dinv * dinv.T + np.eye(C, dtype=np.float32)  # [10,10]

    bd_sizes = sorted({nb * C for nb in chunk_batches})
    OT = O // P
    blob = np.zeros((P, BDW * len(bd_sizes) + 3 * OT), np.float32)
    for i, sz in enumerate(bd_sizes):
        nb = sz // C
        for g in range(nb):
            blob[g * C : (g + 1) * C, BDW * i + g * C : BDW * i + (g + 1) * C] = na.T
    scale = np.asarray(gamma, np.float32) / np.sqrt(np.asarray(moving_var, np.float32) + BN_EPS)
    shift2 = np.asarray(beta, np.float32) - np.asarray(moving_mean, np.float32) * scale - scale
    p0 = BDW * len(bd_sizes)
    blob[:, p0 : p0 + OT] = np.asarray(bias, np.float32).reshape(OT, P).T
    blob[:, p0 + OT : p0 + 2 * OT] = scale.reshape(OT, P).T
    blob[:, p0 + 2 * OT : p0 + 3 * OT] = shift2.reshape(OT, P).T
    return blob


def kernel(x, adjacency, kernel, bias, gamma, beta, moving_mean, moving_var):
    B, C_, F = x.shape
    O = kernel.shape[1]
    assert C_ == C
    assert B % N_CORES == 0
    bl = B // N_CORES
    rows = bl * C

    chunk_batches = (12, 12, 8)
    blob = _host_prep(adjacency, kernel, bias, gamma, beta, moving_mean,
                      moving_var, chunk_batches, O)

    nc = build_nc(rows, F, O, chunk_batches)

    kern_np = np.ascontiguousarray(np.asarray(kernel, np.float32))
    x_np = np.asarray(x, np.float32)
    in_maps = []
    for c in range(N_CORES):
        in_maps.append({
            "x_local": np.ascontiguousarray(x_np[c * bl : (c + 1) * bl].reshape(rows, F)),
            "kern": kern_np,
            "blob": blob,
        })

    res = run_bass_kernel_spmd(nc, in_maps, core_ids=list(range(N_CORES)), trace=False)

    out = np.empty((B, C, O), np.float32)
    for c in range(N_CORES):
        outT = res.results[c]["outT"]  # [O, rows]
        out[c * bl : (c + 1) * bl] = outT.T.reshape(bl, C, O)
    return out



# revision 3
# speedup vs baseline: 1.6831x; 1.6831x over previous
"""GNN message-passing kernel for Trainium2 (8 NeuronCores, batch-parallel).

Computation (per reference):
    norm_adj = adjacency * dinv * dinv.T + I            [10,10]   (host, O(100) flops)
    support  = einsum('bcf,fo->bco', x, kernel)         [B,C,O]
    out      = elu(einsum('ij,bjo->bio', norm_adj, support) + bias)
    out      = (out - mean) * rsqrt(var+eps) * gamma + beta

Strategy: the channel mix commutes with the dense matmul
(norm_adj @ (x @ K) == (norm_adj @ x) @ K), and is only ~0.5% of the
FLOPs, so the host pre-mixes y = norm_adj @ x, pre-transposes it to the
[f, rows] layout the PE needs, and ships it in bf16 (half the DMA, full
PE rate, ~1e-3 relative error).  Each core then runs a single streaming
matmul at the fp22 roofline:

  outT[o, rows] += K[f,o].T @ yT[f, rows]    (bf16 x bf16 -> fp32 PSUM)

with a fused epilogue on ACT/DVE with per-partition (o) params:
  elu(z) = min(exp(z), relu(z)+1) - 1 (exact), then folded BN affine.
Output is stored transposed [O, rows] in bf16; host casts/transposes
while unsharding.
"""

from contextlib import ExitStack

import numpy as np
import ml_dtypes

import concourse.bass as bass
import concourse.bacc as bacc
import concourse.mybir as mybir
import concourse.tile as tile
from concourse.bass_utils import run_bass_kernel_spmd

F32 = mybir.dt.float32
BF16 = mybir.dt.bfloat16
NP_BF16 = ml_dtypes.bfloat16
ALU = mybir.AluOpType
ACTF = mybir.ActivationFunctionType

P = 128
BN_EPS = 1e-3
N_CORES = 8
C = 10  # channels


def build_nc(rows, F, O, panel=512, n_cores=N_CORES, repeats=1):
    """Build the per-core Bass program. rows = local (b,c) rows, F/O = feat dims.

    repeats>1 replays the whole computation (for timing amplification)."""
    assert rows % panel == 0
    n_panels = rows // panel
    FT, OT = F // P, O // P

    nc = bacc.Bacc(
        "TRN2",
        target_bir_lowering=False,
        debug=False,
        enable_asserts=False,
        num_devices=n_cores,
    )
    # yt packs the pre-mixed, pre-transposed activations: yt[p, fb, r] =
    # y[r, fb*128+p] so a panel slice is one strided DMA into SBUF layout.
    yt_d = nc.dram_tensor("yt", [P, FT, rows], BF16, kind="ExternalInput").ap()
    k_d = nc.dram_tensor("kern", [F, O], BF16, kind="ExternalInput").ap()
    # blob cols: [0:OT]=bias_t, [OT:2OT]=scale_t, [2OT:3OT]=shift2_t (per-partition o)
    blob_d = nc.dram_tensor("blob", [P, 3 * OT], F32, kind="ExternalInput").ap()
    outT_d = nc.dram_tensor("outT", [O, rows], BF16, kind="ExternalOutput").ap()

    with tile.TileContext(nc) as tc, ExitStack() as ctx:
        const = ctx.enter_context(tc.tile_pool(name="const", bufs=1))
        blob = const.tile([P, 3 * OT], F32, name="blob")
        nc.sync.dma_start(blob, blob_d)
        kb = [const.tile([P, O], BF16, name=f"kb{fb}", tag=f"kb{fb}") for fb in range(FT)]
        for fb in range(FT):
            nc.scalar.dma_start(kb[fb], k_d[fb * P : (fb + 1) * P, :])

        ypool = ctx.enter_context(tc.tile_pool(name="ypool", bufs=2))
        mainps = ctx.enter_context(tc.tile_pool(name="mainps", bufs=4, space="PSUM"))
        tmp = ctx.enter_context(tc.tile_pool(name="tmp", bufs=3))

        for rep in range(repeats):
          for pi in range(n_panels):
            r0 = pi * panel
            yt = ypool.tile([P, FT, panel], BF16, name=f"r{rep}_y{pi}", tag="yt")
            nc.sync.dma_start(yt, yt_d[:, :, r0 : r0 + panel])
            for ot in range(OT):
                ps = mainps.tile([P, panel], F32, name=f"r{rep}_ps_{pi}_{ot}", tag="ps")
                for fb in range(FT):
                    nc.tensor.matmul(
                        ps,
                        lhsT=kb[fb][:, ot * P : (ot + 1) * P],
                        rhs=yt[:, fb, :],
                        start=(fb == 0),
                        stop=(fb == FT - 1),
                    )
                bias_ap = blob[:, ot : ot + 1]
                scale_ap = blob[:, OT + ot : OT + ot + 1]
                shift_ap = blob[:, 2 * OT + ot : 2 * OT + ot + 1]
                e = tmp.tile([P, panel], F32, name=f"r{rep}_e_{pi}_{ot}", tag="e")
                t0 = tmp.tile([P, panel], F32, name=f"r{rep}_t_{pi}_{ot}", tag="t")
                s = tmp.tile([P, panel], F32, name=f"r{rep}_s_{pi}_{ot}", tag="s")
                fin = tmp.tile([P, panel], BF16, name=f"r{rep}_f_{pi}_{ot}", tag="f")
                nc.scalar.activation(e, ps, ACTF.Exp, bias=bias_ap)
                nc.scalar.activation(t0, ps, ACTF.Relu, bias=bias_ap)
                # elu(zb) + 1 = min(exp(zb), relu(zb) + 1)   (exact identity)
                nc.vector.scalar_tensor_tensor(
                    s, in0=t0, scalar=1.0, in1=e, op0=ALU.add, op1=ALU.min
                )
                # fin = s*scale + (shift - scale) = elu*scale + shift
                nc.vector.tensor_scalar(
                    fin, s, scale_ap, shift_ap, op0=ALU.mult, op1=ALU.add
                )
                nc.scalar.dma_start(outT_d[ot * P : (ot + 1) * P, r0 : r0 + panel], fin)
    nc.compile()
    return nc


def _host_prep(x, adjacency, kern, bias, gamma, beta, moving_mean, moving_var,
               n_cores=N_CORES):
    """Host-side prep: normalized adjacency mix, transpose/tile/cast to bf16.

    Returns (yt_per_core, kern_bf16, blob)."""
    B, C_, F = x.shape
    O = kern.shape[1]
    assert C_ == C
    bl = B // n_cores
    rows = bl * C
    FT, OT = F // P, O // P

    A = np.asarray(adjacency, np.float32)
    deg = np.maximum(np.abs(A).sum(axis=1, keepdims=True), 1e-8)
    dinv = deg ** -0.5
    na = A * dinv * dinv.T + np.eye(C, dtype=np.float32)  # [10,10]

    x_np = np.asarray(x, np.float32)
    # y[i, b, f] = sum_j na[i,j] x[b,j,f]  -- one sgemm [10,10]@[10,B*F]
    y_ibf = np.tensordot(na, x_np, axes=(1, 1)).astype(NP_BF16)  # [C, B, F]

    yt_per_core = []
    for c in range(n_cores):
        yc = y_ibf[:, c * bl : (c + 1) * bl, :]           # [C, bl, F]
        # yt[p, fb, b*C + i] = y[i, b, fb*128+p]
        yt = np.ascontiguousarray(
            yc.reshape(C, bl, FT, P).transpose(3, 2, 1, 0).reshape(P, FT, rows)
        )
        yt_per_core.append(yt)

    kern_bf16 = np.ascontiguousarray(np.asarray(kern, np.float32).astype(NP_BF16))

    scale = np.asarray(gamma, np.float32) / np.sqrt(
        np.asarray(moving_var, np.float32) + BN_EPS
    )
    shift2 = (
        np.asarray(beta, np.float32)
        - np.asarray(moving_mean, np.float32) * scale
        - scale
    )
    blob = np.zeros((P, 3 * OT), np.float32)
    blob[:, 0:OT] = np.asarray(bias, np.float32).reshape(OT, P).T
    blob[:, OT : 2 * OT] = scale.reshape(OT, P).T
    blob[:, 2 * OT : 3 * OT] = shift2.reshape(OT, P).T
    return yt_per_core, kern_bf16, blob


def kernel(x, adjacency, kernel, bias, gamma, beta, moving_mean, moving_var):
    B, C_, F = x.shape
    O = kernel.shape[1]
    assert C_ == C
    assert B % N_CORES == 0
    bl = B // N_CORES
    rows = bl * C

    yt_per_core, kern_bf16, blob = _host_prep(
        x, adjacency, kernel, bias, gamma, beta, moving_mean, moving_var
    )

    nc = build_nc(rows, F, O)

    in_maps = []
    for c in range(N_CORES):
        in_maps.append({
            "yt": yt_per_core[c],
            "kern": kern_bf16,
            "blob": blob,
        })

    res = run_bass_kernel_spmd(nc, in_maps, core_ids=list(range(N_CORES)), trace=False)

    out = np.empty((B, C, O), np.float32)
    for c in range(N_CORES):
        outT = np.asarray(res.results[c]["outT"]).astype(np.float32)  # [O, rows]
        out[c * bl : (c + 1) * bl] = outT.T.reshape(bl, C, O)
    return out
